# revision 3
# baseline (speedup 1.0000x reference)
"""Trainium2 Bass kernel: conv3x3(64->128) + ReLU + conv3x3(128->128) + ReLU + maxpool2x2.

Input  x: [32, 64, 112, 112] f32; weights w1 [128,64,3,3], w2 [128,128,3,3]; biases [128].
Output: [32, 128, 56, 56] f32.

Strategy: data-parallel over batch across 8 cores (4 images/core). Per image,
channels live on SBUF partitions and spatial positions on the free dim with a
zero-padded 114x114 layout. Each conv tap (ky,kx) is a matmul over channels at
a shifted spatial offset, accumulated in PSUM. Conv1 (K=64) packs two K=64
matmuls in the 128x128 PE array via row-group tile_position (0,0)/(64,0): the
image's top/bottom row-halves are processed concurrently from partition halves
0:64 / 64:128. Conv2 is K=128 full-array. Matmuls run in float32r (TF32-like,
1 cycle/row). ReLU+bias fused in ScalarE PSUM->SBUF copies; maxpool via two
strided VectorE max ops.
"""
import numpy as np

import concourse.bass as bass
import concourse.mybir as mybir
from concourse import bacc
from concourse.tile import TileContext
from concourse.bass_utils import run_bass_kernel_spmd

N_CORES = 8
B, CIN, COUT, H, W = 32, 64, 128, 112, 112
PB = B // N_CORES            # images per core
HP = H + 2                   # padded width/height (114)
G = 128                      # zero guard columns around each padded buffer
RHALF = 58                   # padded rows held per half-region (incl. 1-row halo)
LHALF = RHALF * HP           # 6612
LXS = G + LHALF + G          # x half-region buffer length
LY1 = G + HP * HP + G       # conv1 output (padded) buffer length
NROW = 4                     # output rows per PSUM chunk
NCH = NROW * HP              # matmul free dim per chunk (456)
NR1 = (H // 2) // NROW       # conv1 chunk rounds per half (14)
NR2 = H // NROW              # conv2 chunks (28)
HO, WO = H // 2, W // 2      # pooled output dims

F32 = mybir.dt.float32
F32R = mybir.dt.float32r
RELU = mybir.ActivationFunctionType.Relu

# tap offsets in padded flat coords, tap t = (ky, kx)
TAP_OFF = [(ky - 1) * HP + (kx - 1) for ky in range(3) for kx in range(3)]

_CACHE = {}

TRACE = False          # test harness may flip this for profiled runs
LAST_RESULT = None     # stashes BassKernelResults of the last run


def _build():
    nc = bacc.Bacc("TRN2", target_bir_lowering=False, debug=False,
                   num_devices=N_CORES)
    x = nc.dram_tensor("x", [PB, CIN, H, W], F32, kind="ExternalInput")
    w1t = nc.dram_tensor("w1t", [128, 9 * 128], F32, kind="ExternalInput")
    w2t = nc.dram_tensor("w2t", [128, 9 * 128], F32, kind="ExternalInput")
    b1 = nc.dram_tensor("b1", [128, 1], F32, kind="ExternalInput")
    b2 = nc.dram_tensor("b2", [128, 1], F32, kind="ExternalInput")
    y = nc.dram_tensor("y", [PB, COUT, HO, WO], F32, kind="ExternalOutput")

    with TileContext(nc) as tc:
        with (
            tc.tile_pool(name="const", bufs=1) as cpool,
            tc.tile_pool(name="xs", bufs=1) as xpool,
            tc.tile_pool(name="y1p", bufs=1) as y1pool,
            tc.tile_pool(name="work", bufs=4) as wpool,
            tc.tile_pool(name="oimg", bufs=2) as opool,
            tc.tile_pool(name="psA", bufs=2, space="PSUM") as psApool,
            tc.tile_pool(name="psB", bufs=2, space="PSUM") as psBpool,
            tc.tile_pool(name="psC", bufs=3, space="PSUM") as psCpool,
        ):
            w1sb = cpool.tile([128, 9 * 128], F32R, tag="w1")
            w2sb = cpool.tile([128, 9 * 128], F32R, tag="w2")
            b1sb = cpool.tile([128, 1], F32, tag="b1")
            b2sb = cpool.tile([128, 1], F32, tag="b2")
            nc.gpsimd.dma_start(out=w1sb[:, :], in_=w1t[:, :])
            nc.gpsimd.dma_start(out=w2sb[:, :], in_=w2t[:, :])
            nc.sync.dma_start(out=b1sb[:, :], in_=b1[:, :])
            nc.sync.dma_start(out=b2sb[:, :], in_=b2[:, :])

            # persistent padded buffers; zero once, borders stay zero forever
            xs = [xpool.tile([128, LXS], F32R, tag=f"xs{i}", name=f"xs{i}")
                  for i in range(2)]
            y1 = y1pool.tile([128, LY1], F32R, tag="y1")
            for t in xs:
                nc.gpsimd.memset(t[:, :].bitcast(F32), 0.0)
            nc.gpsimd.memset(y1[:, :].bitcast(F32), 0.0)

            y1v = y1[:, G:G + HP * HP].rearrange("p (r c) -> p r c", c=HP)

            for b in range(PB):
                xsb = xs[b % 2]
                xv = xsb[:, G:G + LHALF].rearrange("p (r c) -> p r c", c=HP)
                # top half: padded rows 0..57 (data rows 0..56 into local 1..57)
                nc.gpsimd.dma_start(out=xv[0:64, 1:58, 1:113],
                                    in_=x[b, :, 0:57, :])
                # bottom half: padded rows 56..113 (data rows 55..111 into local 0..56)
                nc.gpsimd.dma_start(out=xv[64:128, 0:57, 1:113],
                                    in_=x[b, :, 55:112, :])

                # ---- conv1: two concurrent K=64 row-group matmul series ----
                for ri in range(NR1):
                    r = 1 + NROW * ri          # local output row base (both halves)
                    q = G + r * HP
                    psA = psApool.tile([128, NCH], F32, tag="psA")
                    psB = psBpool.tile([128, NCH], F32, tag="psB")
                    for t in range(9):
                        off = TAP_OFF[t]
                        nc.tensor.matmul(psA[:, :],
                                         w1sb[0:64, t * 128:(t + 1) * 128],
                                         xsb[0:64, q + off:q + off + NCH],
                                         start=(t == 0), stop=(t == 8),
                                         tile_position=(0, 0))
                        nc.tensor.matmul(psB[:, :],
                                         w1sb[64:128, t * 128:(t + 1) * 128],
                                         xsb[64:128, q + off:q + off + NCH],
                                         start=(t == 0), stop=(t == 8),
                                         tile_position=(64, 0))
                    pAv = psA.rearrange("p (r c) -> p r c", c=HP)
                    pBv = psB.rearrange("p (r c) -> p r c", c=HP)
                    # top half outputs: padded rows r..r+3; bottom: 56+r..56+r+3
                    nc.scalar.activation(y1v[:, r:r + NROW, 1:113],
                                         pAv[:, :, 1:113], RELU,
                                         bias=b1sb[:, 0:1])
                    nc.scalar.activation(y1v[:, 56 + r:56 + r + NROW, 1:113],
                                         pBv[:, :, 1:113], RELU,
                                         bias=b1sb[:, 0:1])

                # ---- conv2 (K=128) + fused relu + maxpool ----
                out_img = opool.tile([128, HO * WO], F32, tag="oimg")
                for ci in range(NR2):
                    r = 1 + NROW * ci          # padded output row base
                    q = G + r * HP
                    psC = psCpool.tile([128, NCH], F32, tag="psC")
                    for t in range(9):
                        off = TAP_OFF[t]
                        nc.tensor.matmul(psC[:, :],
                                         w2sb[:, t * 128:(t + 1) * 128],
                                         y1[:, q + off:q + off + NCH],
                                         start=(t == 0), stop=(t == 8))
                    y2c = wpool.tile([128, NROW * W], F32, tag="y2c")
                    y2v = y2c.rearrange("p (r c) -> p r c", c=W)
                    pCv = psC.rearrange("p (r c) -> p r c", c=HP)
                    nc.scalar.activation(y2v[:, :, :], pCv[:, :, 1:113], RELU,
                                         bias=b2sb[:, 0:1])
                    # horizontal 2:1 max
                    hpt = wpool.tile([128, NROW * WO], F32, tag="hp")
                    y2p = y2c.rearrange("p (r c two) -> p r c two", two=2, c=WO)
                    nc.vector.tensor_max(
                        hpt.rearrange("p (r c) -> p r c", c=WO),
                        y2p[:, :, :, 0], y2p[:, :, :, 1])
                    # vertical 2:1 max -> 2 pooled rows
                    hpv = hpt.rearrange("p (r two c) -> p r two c", two=2, c=WO)
                    ov = out_img[:, ci * 2 * WO:(ci * 2 + 2) * WO].rearrange(
                        "p (r c) -> p r c", c=WO)
                    nc.vector.tensor_max(ov, hpv[:, :, 0, :], hpv[:, :, 1, :])

                nc.sync.dma_start(out=y[b].rearrange("c h w -> c (h w)"),
                                  in_=out_img[:, :])

    nc.compile()
    return nc


def kernel(x, w1, b1, w2, b2):
    global LAST_RESULT
    x = np.ascontiguousarray(np.asarray(x, dtype=np.float32))
    w1 = np.asarray(w1, dtype=np.float32)
    w2 = np.asarray(w2, dtype=np.float32)
    b1 = np.asarray(b1, dtype=np.float32)
    b2 = np.asarray(b2, dtype=np.float32)

    if "nc" not in _CACHE:
        _CACHE["nc"] = _build()
    nc = _CACHE["nc"]

    # weight layout: w1t[ci, t*128+co] = w1[co, ci, ky, kx]; duplicated on
    # partitions 64:128 for the upper row-group. w2t likewise (full 128 rows).
    w1r = np.transpose(w1, (1, 2, 3, 0)).reshape(CIN, 9 * 128)  # ci,(ky kx co)
    # reorder to (t*128 + co): currently (ky,kx) major over co -> already t-major
    w1full = np.concatenate([w1r, w1r], axis=0)                  # [128, 1152]
    w2r = np.transpose(w2, (1, 2, 3, 0)).reshape(COUT, 9 * 128)

    in_maps = []
    for c in range(N_CORES):
        in_maps.append({
            "x": np.ascontiguousarray(x[c * PB:(c + 1) * PB]),
            "w1t": w1full,
            "w2t": w2r,
            "b1": b1.reshape(128, 1),
            "b2": b2.reshape(128, 1),
        })

    res = run_bass_kernel_spmd(nc, in_maps, core_ids=list(range(N_CORES)),
                               trace=TRACE)
    LAST_RESULT = res
    out = np.empty((B, COUT, HO, WO), dtype=np.float32)
    for c in range(N_CORES):
        out[c * PB:(c + 1) * PB] = res.results[c]["y"]
    return out


# revision 7
# speedup vs baseline: 1.0036x; 1.0036x over previous
"""Trainium2 Bass kernel: conv3x3(64->128) + ReLU + conv3x3(128->128) + ReLU + maxpool2x2.

Input  x: [32, 64, 112, 112] f32; weights w1 [128,64,3,3], w2 [128,128,3,3]; biases [128].
Output: [32, 128, 56, 56] f32.

Strategy: data-parallel over batch across 8 cores (4 images/core). Per image,
channels live on SBUF partitions and spatial positions on the free dim with a
zero-padded 114x114 layout. Each conv tap (ky,kx) is a matmul over channels at
a shifted spatial offset, accumulated in PSUM. Conv1 (K=64) packs two K=64
matmuls in the 128x128 PE array via row-group tile_position (0,0)/(64,0): the
image's top/bottom row-halves are processed concurrently from partition halves
0:64 / 64:128. Conv2 is K=128 full-array. Matmuls run in float32r (TF32-like,
1 cycle/row). ReLU+bias fused in ScalarE PSUM->SBUF copies; maxpool via two
strided VectorE max ops.
"""
import numpy as np

import concourse.bass as bass
import concourse.mybir as mybir
from concourse import bacc
from concourse.tile import TileContext
from concourse.bass_utils import run_bass_kernel_spmd

N_CORES = 8
B, CIN, COUT, H, W = 32, 64, 128, 112, 112
PB = B // N_CORES            # images per core
HP = H + 2                   # padded width/height (114)
G = 128                      # zero guard columns around each padded buffer
RHALF = 58                   # padded rows held per half-region (incl. 1-row halo)
LHALF = RHALF * HP           # 6612
LXS = G + LHALF + G          # x half-region buffer length
LY1 = G + HP * HP + G       # conv1 output (padded) buffer length
NROW = 4                     # output rows per PSUM chunk
NCH = NROW * HP              # matmul free dim per chunk (456)
NR1 = (H // 2) // NROW       # conv1 chunk rounds per half (14)
NR2 = H // NROW              # conv2 chunks (28)
HO, WO = H // 2, W // 2      # pooled output dims

F32 = mybir.dt.float32
F32R = mybir.dt.float32r
RELU = mybir.ActivationFunctionType.Relu

# tap offsets in padded flat coords, tap t = (ky, kx)
TAP_OFF = [(ky - 1) * HP + (kx - 1) for ky in range(3) for kx in range(3)]

_CACHE = {}

TRACE = False          # test harness may flip this for profiled runs
LAST_RESULT = None     # stashes BassKernelResults of the last run


def _build():
    nc = bacc.Bacc("TRN2", target_bir_lowering=False, debug=False,
                   num_devices=N_CORES)
    x = nc.dram_tensor("x", [PB, CIN, H, W], F32, kind="ExternalInput")
    w1t = nc.dram_tensor("w1t", [128, 9 * 128], F32, kind="ExternalInput")
    w2t = nc.dram_tensor("w2t", [128, 9 * 128], F32, kind="ExternalInput")
    b1 = nc.dram_tensor("b1", [128, 1], F32, kind="ExternalInput")
    b2 = nc.dram_tensor("b2", [128, 1], F32, kind="ExternalInput")
    y = nc.dram_tensor("y", [PB, COUT, HO, WO], F32, kind="ExternalOutput")

    with TileContext(nc) as tc:
        with (
            tc.tile_pool(name="const", bufs=1) as cpool,
            tc.tile_pool(name="xs", bufs=1) as xpool,
            tc.tile_pool(name="y1p", bufs=1) as y1pool,
            tc.tile_pool(name="work", bufs=4) as wpool,
            tc.tile_pool(name="oimg", bufs=2) as opool,
            tc.tile_pool(name="psA", bufs=2, space="PSUM") as psApool,
            tc.tile_pool(name="psB", bufs=2, space="PSUM") as psBpool,
            tc.tile_pool(name="psC", bufs=3, space="PSUM") as psCpool,
        ):
            w1sb = cpool.tile([128, 9 * 128], F32R, tag="w1")
            w2sb = cpool.tile([128, 9 * 128], F32R, tag="w2")
            b1sb = cpool.tile([128, 1], F32, tag="b1")
            b2sb = cpool.tile([128, 1], F32, tag="b2")
            nc.gpsimd.dma_start(out=w1sb[:, :], in_=w1t[:, :])
            nc.gpsimd.dma_start(out=w2sb[:, :], in_=w2t[:, :])
            nc.sync.dma_start(out=b1sb[:, :], in_=b1[:, :])
            nc.sync.dma_start(out=b2sb[:, :], in_=b2[:, :])

            # persistent padded buffers; zero the borders once (interior is
            # fully overwritten every image), so borders stay zero forever.
            xs = [xpool.tile([128, LXS], F32R, tag=f"xs{i}", name=f"xs{i}")
                  for i in range(2)]
            y1 = y1pool.tile([128, LY1], F32R, tag="y1")
            for t in xs:
                tv = t[:, :].bitcast(F32)
                # guard fringe actually read: 1 elem each side (use 8)
                nc.gpsimd.memset(tv[:, G - 8:G], 0.0)
                nc.gpsimd.memset(tv[:, G + LHALF:G + LHALF + 8], 0.0)
                # pad row 0 (top halo) and row 57 (bottom halo)
                nc.gpsimd.memset(tv[:, G:G + HP], 0.0)
                nc.gpsimd.memset(tv[:, G + 57 * HP:G + 58 * HP], 0.0)
                # column borders: col 113 of row r + col 0 of row r+1, r=0..56
                cb = tv[:, G + 113:G + 113 + 57 * HP].rearrange(
                    "p (r c) -> p r c", c=HP)
                nc.gpsimd.memset(cb[:, :, 0:2], 0.0)
            y1f = y1[:, :].bitcast(F32)
            nc.vector.memset(y1f[:, G - 8:G], 0.0)
            nc.vector.memset(y1f[:, G + HP * HP:G + HP * HP + 8], 0.0)
            nc.vector.memset(y1f[:, G:G + HP], 0.0)
            nc.vector.memset(y1f[:, G + 113 * HP:G + 114 * HP], 0.0)
            y1cb = y1f[:, G + 113:G + 113 + 113 * HP].rearrange(
                "p (r c) -> p r c", c=HP)
            nc.vector.memset(y1cb[:, :, 0:2], 0.0)

            # PE warmup: zero-weight K=64 matmuls accumulating into the first
            # conv1 PSUM tile while the initial DMAs run, so the PE clock
            # gate (HAM) is at full rate when real matmuls start.
            warm = cpool.tile([128, NCH], F32R, tag="warm")
            nc.vector.memset(warm[:, :].bitcast(F32), 0.0)
            warm_ps = psApool.tile([128, NCH], F32, tag="psA", name="warm_ps")
            N_WARM = 96
            for k in range(N_WARM):
                nc.tensor.matmul(warm_ps[:, :], warm[0:64, 0:128],
                                 warm[0:64, :], start=(k == 0), stop=False,
                                 tile_position=(0, 0))
            # image 0 / round 0 conv1 accumulates on top of the (zero) warmup
            # sums in warm_ps, so the warmup matmuls feed a live output and
            # cannot be dead-code eliminated.

            y1v = y1[:, G:G + HP * HP].rearrange("p (r c) -> p r c", c=HP)

            for b in range(PB):
                xsb = xs[b % 2]
                xv = xsb[:, G:G + LHALF].rearrange("p (r c) -> p r c", c=HP)
                # top half: padded rows 0..57 (data rows 0..56 into local 1..57)
                nc.gpsimd.dma_start(out=xv[0:64, 1:58, 1:113],
                                    in_=x[b, :, 0:57, :])
                # bottom half: padded rows 56..113 (data rows 55..111 into local 0..56)
                nc.gpsimd.dma_start(out=xv[64:128, 0:57, 1:113],
                                    in_=x[b, :, 55:112, :])

                # ---- conv1: two concurrent K=64 row-group matmul series ----
                for ri in range(NR1):
                    r = 1 + NROW * ri          # local output row base (both halves)
                    q = G + r * HP
                    warm_round = (b == 0 and ri == 0)
                    if warm_round:
                        psA = warm_ps          # continue warmup accumulation
                    else:
                        psA = psApool.tile([128, NCH], F32, tag="psA")
                    psB = psBpool.tile([128, NCH], F32, tag="psB")
                    for t in range(9):
                        off = TAP_OFF[t]
                        nc.tensor.matmul(psA[:, :],
                                         w1sb[0:64, t * 128:(t + 1) * 128],
                                         xsb[0:64, q + off:q + off + NCH],
                                         start=(t == 0 and not warm_round),
                                         stop=(t == 8),
                                         tile_position=(0, 0))
                        nc.tensor.matmul(psB[:, :],
                                         w1sb[64:128, t * 128:(t + 1) * 128],
                                         xsb[64:128, q + off:q + off + NCH],
                                         start=(t == 0), stop=(t == 8),
                                         tile_position=(64, 0))
                    pAv = psA.rearrange("p (r c) -> p r c", c=HP)
                    pBv = psB.rearrange("p (r c) -> p r c", c=HP)
                    # top half outputs: padded rows r..r+3; bottom: 56+r..56+r+3
                    nc.scalar.activation(y1v[:, r:r + NROW, 1:113],
                                         pAv[:, :, 1:113], RELU,
                                         bias=b1sb[:, 0:1])
                    nc.scalar.activation(y1v[:, 56 + r:56 + r + NROW, 1:113],
                                         pBv[:, :, 1:113], RELU,
                                         bias=b1sb[:, 0:1])

                # ---- conv2 (K=128) + fused relu + maxpool ----
                out_img = opool.tile([128, HO * WO], F32, tag="oimg")
                for ci in range(NR2):
                    r = 1 + NROW * ci          # padded output row base
                    q = G + r * HP
                    psC = psCpool.tile([128, NCH], F32, tag="psC")
                    for t in range(9):
                        off = TAP_OFF[t]
                        nc.tensor.matmul(psC[:, :],
                                         w2sb[:, t * 128:(t + 1) * 128],
                                         y1[:, q + off:q + off + NCH],
                                         start=(t == 0), stop=(t == 8))
                    y2c = wpool.tile([128, NROW * W], F32, tag="y2c")
                    y2v = y2c.rearrange("p (r c) -> p r c", c=W)
                    pCv = psC.rearrange("p (r c) -> p r c", c=HP)
                    nc.scalar.activation(y2v[:, :, :], pCv[:, :, 1:113], RELU,
                                         bias=b2sb[:, 0:1])
                    # horizontal 2:1 max
                    hpt = wpool.tile([128, NROW * WO], F32, tag="hp")
                    y2p = y2c.rearrange("p (r c two) -> p r c two", two=2, c=WO)
                    nc.vector.tensor_max(
                        hpt.rearrange("p (r c) -> p r c", c=WO),
                        y2p[:, :, :, 0], y2p[:, :, :, 1])
                    # vertical 2:1 max -> 2 pooled rows
                    hpv = hpt.rearrange("p (r two c) -> p r two c", two=2, c=WO)
                    ov = out_img[:, ci * 2 * WO:(ci * 2 + 2) * WO].rearrange(
                        "p (r c) -> p r c", c=WO)
                    nc.vector.tensor_max(ov, hpv[:, :, 0, :], hpv[:, :, 1, :])

                nc.sync.dma_start(out=y[b].rearrange("c h w -> c (h w)"),
                                  in_=out_img[:, :])

    nc.compile()
    return nc


def kernel(x, w1, b1, w2, b2):
    global LAST_RESULT
    x = np.ascontiguousarray(np.asarray(x, dtype=np.float32))
    w1 = np.asarray(w1, dtype=np.float32)
    w2 = np.asarray(w2, dtype=np.float32)
    b1 = np.asarray(b1, dtype=np.float32)
    b2 = np.asarray(b2, dtype=np.float32)

    if "nc" not in _CACHE:
        _CACHE["nc"] = _build()
    nc = _CACHE["nc"]

    # weight layout: w1t[ci, t*128+co] = w1[co, ci, ky, kx]; duplicated on
    # partitions 64:128 for the upper row-group. w2t likewise (full 128 rows).
    w1r = np.transpose(w1, (1, 2, 3, 0)).reshape(CIN, 9 * 128)  # ci,(ky kx co)
    # reorder to (t*128 + co): currently (ky,kx) major over co -> already t-major
    w1full = np.concatenate([w1r, w1r], axis=0)                  # [128, 1152]
    w2r = np.transpose(w2, (1, 2, 3, 0)).reshape(COUT, 9 * 128)

    in_maps = []
    for c in range(N_CORES):
        in_maps.append({
            "x": np.ascontiguousarray(x[c * PB:(c + 1) * PB]),
            "w1t": w1full,
            "w2t": w2r,
            "b1": b1.reshape(128, 1),
            "b2": b2.reshape(128, 1),
        })

    res = run_bass_kernel_spmd(nc, in_maps, core_ids=list(range(N_CORES)),
                               trace=TRACE)
    LAST_RESULT = res
    out = np.empty((B, COUT, HO, WO), dtype=np.float32)
    for c in range(N_CORES):
        out[c * PB:(c + 1) * PB] = res.results[c]["y"]
    return out


# revision 10
# speedup vs baseline: 1.0673x; 1.0635x over previous
"""Trainium2 Bass kernel: conv3x3(64->128) + ReLU + conv3x3(128->128) + ReLU + maxpool2x2.

Input  x: [32, 64, 112, 112] f32; weights w1 [128,64,3,3], w2 [128,128,3,3]; biases [128].
Output: [32, 128, 56, 56] f32.

Strategy: data-parallel over batch across 8 cores (4 images/core). Per image,
channels live on SBUF partitions and spatial positions on the free dim with a
zero-padded 114x114 layout. Each conv tap (ky,kx) is a matmul over channels at
a shifted spatial offset, accumulated in PSUM. Conv1 (K=64) packs two K=64
matmuls in the 128x128 PE array via row-group tile_position (0,0)/(64,0): the
image's top/bottom row-halves are processed concurrently from partition halves
0:64 / 64:128. Conv2 is K=128 full-array. Matmuls run in float32r (TF32-like,
1 cycle/row). ReLU+bias fused in ScalarE PSUM->SBUF copies; maxpool via two
strided VectorE max ops.
"""
import numpy as np

import concourse.bass as bass
import concourse.mybir as mybir
from concourse import bacc
from concourse.tile import TileContext
from concourse.bass_utils import run_bass_kernel_spmd

N_CORES = 8
B, CIN, COUT, H, W = 32, 64, 128, 112, 112
PB = B // N_CORES            # images per core
HP = H + 2                   # padded width/height (114)
G = 128                      # zero guard columns around each padded buffer
RHALF = 58                   # padded rows held per half-region (incl. 1-row halo)
LHALF = RHALF * HP           # 6612
LXS = G + LHALF + G          # x half-region buffer length
LY1 = G + HP * HP + G       # conv1 output (padded) buffer length
NROW = 4                     # output rows per PSUM chunk
NCH = NROW * HP              # matmul free dim per chunk (456)
NR1 = (H // 2) // NROW       # conv1 chunk rounds per half (14)
NR2 = H // NROW              # conv2 chunks (28)
HO, WO = H // 2, W // 2      # pooled output dims

F32 = mybir.dt.float32
F32R = mybir.dt.float32r
RELU = mybir.ActivationFunctionType.Relu

# tap offsets in padded flat coords, tap t = (ky, kx)
TAP_OFF = [(ky - 1) * HP + (kx - 1) for ky in range(3) for kx in range(3)]

_CACHE = {}

TRACE = False          # test harness may flip this for profiled runs
LAST_RESULT = None     # stashes BassKernelResults of the last run


def _build():
    nc = bacc.Bacc("TRN2", target_bir_lowering=False, debug=False,
                   num_devices=N_CORES, num_swdge_queues=4)
    x = nc.dram_tensor("x", [PB, CIN, H, W], F32, kind="ExternalInput")
    w1t = nc.dram_tensor("w1t", [128, 9 * 128], F32, kind="ExternalInput")
    w2t = nc.dram_tensor("w2t", [128, 9 * 128], F32, kind="ExternalInput")
    b1 = nc.dram_tensor("b1", [128, 1], F32, kind="ExternalInput")
    b2 = nc.dram_tensor("b2", [128, 1], F32, kind="ExternalInput")
    y = nc.dram_tensor("y", [PB, COUT, HO, WO], F32, kind="ExternalOutput")

    with TileContext(nc) as tc:
        with (
            tc.tile_pool(name="const", bufs=1) as cpool,
            tc.tile_pool(name="xs", bufs=1) as xpool,
            tc.tile_pool(name="y1p", bufs=1) as y1pool,
            tc.tile_pool(name="work", bufs=4) as wpool,
            tc.tile_pool(name="oimg", bufs=2) as opool,
            tc.tile_pool(name="psA", bufs=2, space="PSUM") as psApool,
            tc.tile_pool(name="psB", bufs=2, space="PSUM") as psBpool,
            tc.tile_pool(name="psC", bufs=3, space="PSUM") as psCpool,
        ):
            w1sb = cpool.tile([128, 9 * 128], F32R, tag="w1")
            w2sb = cpool.tile([128, 9 * 128], F32R, tag="w2")
            b1sb = cpool.tile([128, 1], F32, tag="b1")
            b2sb = cpool.tile([128, 1], F32, tag="b2")
            nc.gpsimd.dma_start(out=w1sb[:, :], in_=w1t[:, :])
            nc.gpsimd.dma_start(out=w2sb[:, :], in_=w2t[:, :])
            nc.sync.dma_start(out=b1sb[:, :], in_=b1[:, :])
            nc.sync.dma_start(out=b2sb[:, :], in_=b2[:, :])

            # persistent padded buffers; zero the borders once (interior is
            # fully overwritten every image), so borders stay zero forever.
            xs = [xpool.tile([128, LXS], F32R, tag=f"xs{i}", name=f"xs{i}")
                  for i in range(2)]
            y1 = y1pool.tile([128, LY1], F32R, tag="y1")
            for t in xs:
                tv = t[:, :].bitcast(F32)
                # guard fringe actually read: 1 elem each side (use 8)
                nc.gpsimd.memset(tv[:, G - 8:G], 0.0)
                nc.gpsimd.memset(tv[:, G + LHALF:G + LHALF + 8], 0.0)
                # pad row 0 (top halo) and row 57 (bottom halo)
                nc.gpsimd.memset(tv[:, G:G + HP], 0.0)
                nc.gpsimd.memset(tv[:, G + 57 * HP:G + 58 * HP], 0.0)
                # column borders: col 113 of row r + col 0 of row r+1, r=0..56
                cb = tv[:, G + 113:G + 113 + 57 * HP].rearrange(
                    "p (r c) -> p r c", c=HP)
                nc.gpsimd.memset(cb[:, :, 0:2], 0.0)
            y1f = y1[:, :].bitcast(F32)
            nc.vector.memset(y1f[:, G - 8:G], 0.0)
            nc.vector.memset(y1f[:, G + HP * HP:G + HP * HP + 8], 0.0)
            nc.vector.memset(y1f[:, G:G + HP], 0.0)
            nc.vector.memset(y1f[:, G + 113 * HP:G + 114 * HP], 0.0)
            y1cb = y1f[:, G + 113:G + 113 + 113 * HP].rearrange(
                "p (r c) -> p r c", c=HP)
            nc.vector.memset(y1cb[:, :, 0:2], 0.0)

            # PE warmup: zero-weight K=64 matmuls accumulating into the first
            # conv1 PSUM tile while the initial DMAs run, so the PE clock
            # gate (HAM) is at full rate when real matmuls start.
            warm = cpool.tile([128, NCH], F32R, tag="warm")
            nc.vector.memset(warm[:, :].bitcast(F32), 0.0)
            warm_ps = psApool.tile([128, NCH], F32, tag="psA", name="warm_ps")
            N_WARM = 10
            for k in range(N_WARM):
                nc.tensor.matmul(warm_ps[:, :], warm[0:64, 0:128],
                                 warm[0:64, :], start=(k == 0), stop=False,
                                 tile_position=(0, 0))
            # image 0 / round 0 conv1 accumulates on top of the (zero) warmup
            # sums in warm_ps, so the warmup matmuls feed a live output and
            # cannot be dead-code eliminated.

            y1v = y1[:, G:G + HP * HP].rearrange("p (r c) -> p r c", c=HP)

            for b in range(PB):
                xsb = xs[b % 2]
                xv = xsb[:, G:G + LHALF].rearrange("p (r c) -> p r c", c=HP)
                # top half: padded rows 0..57 (data rows 0..56 into local 1..57)
                # bottom half: padded rows 56..113 (data rows 55..111 into
                # local 0..56); split into row blocks to spread across SWDGE
                # queues so the first image's load completes sooner.
                nc.gpsimd.dma_start(out=xv[0:64, 1:30, 1:113],
                                    in_=x[b, :, 0:29, :])
                nc.gpsimd.dma_start(out=xv[0:64, 30:58, 1:113],
                                    in_=x[b, :, 29:57, :])
                nc.gpsimd.dma_start(out=xv[64:128, 0:29, 1:113],
                                    in_=x[b, :, 55:84, :])
                nc.gpsimd.dma_start(out=xv[64:128, 29:57, 1:113],
                                    in_=x[b, :, 84:112, :])

                # ---- conv1: two concurrent K=64 row-group matmul series ----
                for ri in range(NR1):
                    r = 1 + NROW * ri          # local output row base (both halves)
                    q = G + r * HP
                    warm_round = (b == 0 and ri == 0)
                    if warm_round:
                        psA = warm_ps          # continue warmup accumulation
                    else:
                        psA = psApool.tile([128, NCH], F32, tag="psA")
                    psB = psBpool.tile([128, NCH], F32, tag="psB")
                    for t in range(9):
                        off = TAP_OFF[t]
                        nc.tensor.matmul(psA[:, :],
                                         w1sb[0:64, t * 128:(t + 1) * 128],
                                         xsb[0:64, q + off:q + off + NCH],
                                         start=(t == 0 and not warm_round),
                                         stop=(t == 8),
                                         tile_position=(0, 0))
                        nc.tensor.matmul(psB[:, :],
                                         w1sb[64:128, t * 128:(t + 1) * 128],
                                         xsb[64:128, q + off:q + off + NCH],
                                         start=(t == 0), stop=(t == 8),
                                         tile_position=(64, 0))
                    pAv = psA.rearrange("p (r c) -> p r c", c=HP)
                    pBv = psB.rearrange("p (r c) -> p r c", c=HP)
                    # top half outputs: padded rows r..r+3; bottom: 56+r..56+r+3
                    nc.scalar.activation(y1v[:, r:r + NROW, 1:113],
                                         pAv[:, :, 1:113], RELU,
                                         bias=b1sb[:, 0:1])
                    nc.scalar.activation(y1v[:, 56 + r:56 + r + NROW, 1:113],
                                         pBv[:, :, 1:113], RELU,
                                         bias=b1sb[:, 0:1])

                # ---- conv2 (K=128) + fused relu + maxpool ----
                out_img = opool.tile([128, HO * WO], F32, tag="oimg")
                for ci in range(NR2):
                    r = 1 + NROW * ci          # padded output row base
                    q = G + r * HP
                    psC = psCpool.tile([128, NCH], F32, tag="psC")
                    for t in range(9):
                        off = TAP_OFF[t]
                        nc.tensor.matmul(psC[:, :],
                                         w2sb[:, t * 128:(t + 1) * 128],
                                         y1[:, q + off:q + off + NCH],
                                         start=(t == 0), stop=(t == 8))
                    y2c = wpool.tile([128, NROW * W], F32, tag="y2c")
                    y2v = y2c.rearrange("p (r c) -> p r c", c=W)
                    pCv = psC.rearrange("p (r c) -> p r c", c=HP)
                    nc.scalar.activation(y2v[:, :, :], pCv[:, :, 1:113], RELU,
                                         bias=b2sb[:, 0:1])
                    # horizontal 2:1 max
                    hpt = wpool.tile([128, NROW * WO], F32, tag="hp")
                    y2p = y2c.rearrange("p (r c two) -> p r c two", two=2, c=WO)
                    nc.vector.tensor_max(
                        hpt.rearrange("p (r c) -> p r c", c=WO),
                        y2p[:, :, :, 0], y2p[:, :, :, 1])
                    # vertical 2:1 max -> 2 pooled rows
                    hpv = hpt.rearrange("p (r two c) -> p r two c", two=2, c=WO)
                    ov = out_img[:, ci * 2 * WO:(ci * 2 + 2) * WO].rearrange(
                        "p (r c) -> p r c", c=WO)
                    nc.vector.tensor_max(ov, hpv[:, :, 0, :], hpv[:, :, 1, :])

                nc.sync.dma_start(out=y[b].rearrange("c h w -> c (h w)"),
                                  in_=out_img[:, :])

    nc.compile()
    return nc


def kernel(x, w1, b1, w2, b2):
    global LAST_RESULT
    x = np.ascontiguousarray(np.asarray(x, dtype=np.float32))
    w1 = np.asarray(w1, dtype=np.float32)
    w2 = np.asarray(w2, dtype=np.float32)
    b1 = np.asarray(b1, dtype=np.float32)
    b2 = np.asarray(b2, dtype=np.float32)

    if "nc" not in _CACHE:
        _CACHE["nc"] = _build()
    nc = _CACHE["nc"]

    # weight layout: w1t[ci, t*128+co] = w1[co, ci, ky, kx]; duplicated on
    # partitions 64:128 for the upper row-group. w2t likewise (full 128 rows).
    w1r = np.transpose(w1, (1, 2, 3, 0)).reshape(CIN, 9 * 128)  # ci,(ky kx co)
    # reorder to (t*128 + co): currently (ky,kx) major over co -> already t-major
    w1full = np.concatenate([w1r, w1r], axis=0)                  # [128, 1152]
    w2r = np.transpose(w2, (1, 2, 3, 0)).reshape(COUT, 9 * 128)

    in_maps = []
    for c in range(N_CORES):
        in_maps.append({
            "x": np.ascontiguousarray(x[c * PB:(c + 1) * PB]),
            "w1t": w1full,
            "w2t": w2r,
            "b1": b1.reshape(128, 1),
            "b2": b2.reshape(128, 1),
        })

    res = run_bass_kernel_spmd(nc, in_maps, core_ids=list(range(N_CORES)),
                               trace=TRACE)
    LAST_RESULT = res
    out = np.empty((B, COUT, HO, WO), dtype=np.float32)
    for c in range(N_CORES):
        out[c * PB:(c + 1) * PB] = res.results[c]["y"]
    return out


# revision 13
# speedup vs baseline: 1.1654x; 1.0918x over previous
"""Trainium2 Bass kernel: conv3x3(64->128) + ReLU + conv3x3(128->128) + ReLU + maxpool2x2.

Input  x: [32, 64, 112, 112] f32; weights w1 [128,64,3,3], w2 [128,128,3,3]; biases [128].
Output: [32, 128, 56, 56] f32.

Strategy: data-parallel over batch across 8 cores (4 images/core). Per image,
channels live on SBUF partitions and spatial positions on the free dim with a
zero-padded 114x114 layout. Each conv tap (ky,kx) is a matmul over channels at
a shifted spatial offset, accumulated in PSUM. Conv1 (K=64) packs two K=64
matmuls in the 128x128 PE array via row-group tile_position (0,0)/(64,0): the
image's top/bottom row-halves are processed concurrently from partition halves
0:64 / 64:128. Conv2 is K=128 full-array. Matmuls run in float32r (TF32-like,
1 cycle/row). ReLU+bias fused in ScalarE PSUM->SBUF copies; maxpool via two
strided VectorE max ops.
"""
import numpy as np

import concourse.bass as bass
import concourse.mybir as mybir
from concourse import bacc
from concourse.tile import TileContext
from concourse.bass_utils import run_bass_kernel_spmd

N_CORES = 8
B, CIN, COUT, H, W = 32, 64, 128, 112, 112
PB = B // N_CORES            # images per core
HP = H + 2                   # padded width/height (114)
G = 128                      # zero guard columns around each padded buffer
RHALF = 58                   # padded rows held per half-region (incl. 1-row halo)
LHALF = RHALF * HP           # 6612
LXS = G + LHALF + G          # x half-region buffer length
LY1 = G + HP * HP + G       # conv1 output (padded) buffer length
NROW = 4                     # output rows per PSUM chunk
NCH = NROW * HP              # matmul free dim per chunk (456)
NR1 = (H // 2) // NROW       # conv1 chunk rounds per half (14)
NR2 = H // NROW              # conv2 chunks (28)
HO, WO = H // 2, W // 2      # pooled output dims

F32 = mybir.dt.float32
F32R = mybir.dt.float32r
RELU = mybir.ActivationFunctionType.Relu

# tap offsets in padded flat coords, tap t = (ky, kx)
TAP_OFF = [(ky - 1) * HP + (kx - 1) for ky in range(3) for kx in range(3)]

_CACHE = {}

TRACE = False          # test harness may flip this for profiled runs
LAST_RESULT = None     # stashes BassKernelResults of the last run


def _build():
    nc = bacc.Bacc("TRN2", target_bir_lowering=False, debug=False,
                   num_devices=N_CORES, num_swdge_queues=4)
    x = nc.dram_tensor("x", [PB, CIN, H, W], F32, kind="ExternalInput")
    w1t = nc.dram_tensor("w1t", [128, 9 * 128], F32, kind="ExternalInput")
    w2t = nc.dram_tensor("w2t", [128, 9 * 128], F32, kind="ExternalInput")
    b1 = nc.dram_tensor("b1", [128, 1], F32, kind="ExternalInput")
    b2 = nc.dram_tensor("b2", [128, 1], F32, kind="ExternalInput")
    y = nc.dram_tensor("y", [PB, COUT, HO, WO], F32, kind="ExternalOutput")

    with TileContext(nc) as tc:
        with (
            tc.tile_pool(name="const", bufs=1) as cpool,
            tc.tile_pool(name="xs", bufs=1) as xpool,
            tc.tile_pool(name="y1p", bufs=1) as y1pool,
            tc.tile_pool(name="work", bufs=4) as wpool,
            tc.tile_pool(name="oimg", bufs=2) as opool,
            tc.tile_pool(name="psA", bufs=2, space="PSUM") as psApool,
            tc.tile_pool(name="psB", bufs=2, space="PSUM") as psBpool,
            tc.tile_pool(name="psC", bufs=3, space="PSUM") as psCpool,
        ):
            w1sb = cpool.tile([128, 9 * 128], F32R, tag="w1")
            w2sb = cpool.tile([128, 9 * 128], F32R, tag="w2")
            b1sb = cpool.tile([128, 1], F32, tag="b1")
            b2sb = cpool.tile([128, 1], F32, tag="b2")
            nc.gpsimd.dma_start(out=w1sb[:, :], in_=w1t[:, :])
            nc.gpsimd.dma_start(out=w2sb[:, :], in_=w2t[:, :])
            nc.sync.dma_start(out=b1sb[:, :], in_=b1[:, :])
            nc.sync.dma_start(out=b2sb[:, :], in_=b2[:, :])

            # persistent padded buffers; zero the borders once (interior is
            # fully overwritten every image), so borders stay zero forever.
            xs = [xpool.tile([128, LXS], F32R, tag=f"xs{i}", name=f"xs{i}")
                  for i in range(2)]
            y1 = y1pool.tile([128, LY1], F32R, tag="y1")
            for t in xs:
                tv = t[:, :].bitcast(F32)
                # guard fringe actually read: 1 elem each side (use 8)
                nc.gpsimd.memset(tv[:, G - 8:G], 0.0)
                nc.gpsimd.memset(tv[:, G + LHALF:G + LHALF + 8], 0.0)
                # pad row 0 (top halo) and row 57 (bottom halo)
                nc.gpsimd.memset(tv[:, G:G + HP], 0.0)
                nc.gpsimd.memset(tv[:, G + 57 * HP:G + 58 * HP], 0.0)
                # column borders: col 113 of row r + col 0 of row r+1, r=0..56
                cb = tv[:, G + 113:G + 113 + 57 * HP].rearrange(
                    "p (r c) -> p r c", c=HP)
                nc.gpsimd.memset(cb[:, :, 0:2], 0.0)
            y1f = y1[:, :].bitcast(F32)
            nc.vector.memset(y1f[:, G - 8:G], 0.0)
            nc.vector.memset(y1f[:, G + HP * HP:G + HP * HP + 8], 0.0)
            nc.vector.memset(y1f[:, G:G + HP], 0.0)
            nc.vector.memset(y1f[:, G + 113 * HP:G + 114 * HP], 0.0)
            y1cb = y1f[:, G + 113:G + 113 + 113 * HP].rearrange(
                "p (r c) -> p r c", c=HP)
            nc.vector.memset(y1cb[:, :, 0:2], 0.0)

            # PE warmup: zero-weight K=64 matmuls accumulating into the first
            # conv1 PSUM tile while the initial DMAs run, so the PE clock
            # gate (HAM) is at full rate when real matmuls start.
            warm = cpool.tile([128, NCH], F32R, tag="warm")
            nc.vector.memset(warm[:, :].bitcast(F32), 0.0)
            warm_ps = psApool.tile([128, NCH], F32, tag="psA", name="warm_ps")
            N_WARM = 28
            for k in range(N_WARM):
                nc.tensor.matmul(warm_ps[:, :], warm[0:64, 0:128],
                                 warm[0:64, :], start=(k == 0), stop=False,
                                 tile_position=(0, 0))
            # image 0 / round 0 conv1 accumulates on top of the (zero) warmup
            # sums in warm_ps, so the warmup matmuls feed a live output and
            # cannot be dead-code eliminated.

            y1v = y1[:, G:G + HP * HP].rearrange("p (r c) -> p r c", c=HP)

            for b in range(PB):
                xsb = xs[b % 2]
                xv = xsb[:, G:G + LHALF].rearrange("p (r c) -> p r c", c=HP)
                # top half: padded rows 0..57 (data rows 0..56 into local 1..57)
                # bottom half: padded rows 56..113 (data rows 55..111 into
                # local 0..56); split into row blocks to spread across SWDGE
                # queues so the first image's load completes sooner.
                # first conv1 round only needs padded rows 0..5; load those
                # first so it can start before the bulk arrives
                nc.gpsimd.dma_start(out=xv[0:64, 1:7, 1:113],
                                    in_=x[b, :, 0:6, :])
                nc.gpsimd.dma_start(out=xv[64:128, 0:7, 1:113],
                                    in_=x[b, :, 55:62, :])
                nc.gpsimd.dma_start(out=xv[0:64, 7:30, 1:113],
                                    in_=x[b, :, 6:29, :])
                nc.gpsimd.dma_start(out=xv[0:64, 30:58, 1:113],
                                    in_=x[b, :, 29:57, :])
                nc.gpsimd.dma_start(out=xv[64:128, 7:29, 1:113],
                                    in_=x[b, :, 62:84, :])
                nc.gpsimd.dma_start(out=xv[64:128, 29:57, 1:113],
                                    in_=x[b, :, 84:112, :])

                # ---- conv1: two concurrent K=64 row-group matmul series ----
                for ri in range(NR1):
                    r = 1 + NROW * ri          # local output row base (both halves)
                    q = G + r * HP
                    warm_round = (b == 0 and ri == 0)
                    if warm_round:
                        psA = warm_ps          # continue warmup accumulation
                    else:
                        psA = psApool.tile([128, NCH], F32, tag="psA")
                    psB = psBpool.tile([128, NCH], F32, tag="psB")
                    for t in range(9):
                        off = TAP_OFF[t]
                        nc.tensor.matmul(psA[:, :],
                                         w1sb[0:64, t * 128:(t + 1) * 128],
                                         xsb[0:64, q + off:q + off + NCH],
                                         start=(t == 0 and not warm_round),
                                         stop=(t == 8),
                                         tile_position=(0, 0))
                        nc.tensor.matmul(psB[:, :],
                                         w1sb[64:128, t * 128:(t + 1) * 128],
                                         xsb[64:128, q + off:q + off + NCH],
                                         start=(t == 0), stop=(t == 8),
                                         tile_position=(64, 0))
                    pAv = psA.rearrange("p (r c) -> p r c", c=HP)
                    pBv = psB.rearrange("p (r c) -> p r c", c=HP)
                    # top half outputs: padded rows r..r+3; bottom: 56+r..56+r+3
                    nc.scalar.activation(y1v[:, r:r + NROW, 1:113],
                                         pAv[:, :, 1:113], RELU,
                                         bias=b1sb[:, 0:1])
                    nc.scalar.activation(y1v[:, 56 + r:56 + r + NROW, 1:113],
                                         pBv[:, :, 1:113], RELU,
                                         bias=b1sb[:, 0:1])

                # ---- conv2 (K=128) + fused relu + maxpool ----
                out_img = opool.tile([128, HO * WO], F32, tag="oimg")
                for ci in range(NR2):
                    r = 1 + NROW * ci          # padded output row base
                    q = G + r * HP
                    psC = psCpool.tile([128, NCH], F32, tag="psC")
                    for t in range(9):
                        off = TAP_OFF[t]
                        nc.tensor.matmul(psC[:, :],
                                         w2sb[:, t * 128:(t + 1) * 128],
                                         y1[:, q + off:q + off + NCH],
                                         start=(t == 0), stop=(t == 8))
                    y2c = wpool.tile([128, NROW * W], F32, tag="y2c")
                    y2v = y2c.rearrange("p (r c) -> p r c", c=W)
                    pCv = psC.rearrange("p (r c) -> p r c", c=HP)
                    nc.scalar.activation(y2v[:, :, :], pCv[:, :, 1:113], RELU,
                                         bias=b2sb[:, 0:1])
                    # horizontal 2:1 max
                    hpt = wpool.tile([128, NROW * WO], F32, tag="hp")
                    y2p = y2c.rearrange("p (r c two) -> p r c two", two=2, c=WO)
                    nc.vector.tensor_max(
                        hpt.rearrange("p (r c) -> p r c", c=WO),
                        y2p[:, :, :, 0], y2p[:, :, :, 1])
                    # vertical 2:1 max -> 2 pooled rows
                    hpv = hpt.rearrange("p (r two c) -> p r two c", two=2, c=WO)
                    ov = out_img[:, ci * 2 * WO:(ci * 2 + 2) * WO].rearrange(
                        "p (r c) -> p r c", c=WO)
                    nc.vector.tensor_max(ov, hpv[:, :, 0, :], hpv[:, :, 1, :])

                    if ci % 7 == 6:
                        # stream pooled rows out in 4 slabs per image so the
                        # final slab's DMA is short at kernel tail
                        lo, hi = (ci - 6) * 2 * WO, (ci + 1) * 2 * WO
                        nc.sync.dma_start(
                            out=y[b].rearrange("c h w -> c (h w)")[:, lo:hi],
                            in_=out_img[:, lo:hi])

    nc.compile()
    return nc


def kernel(x, w1, b1, w2, b2):
    global LAST_RESULT
    x = np.ascontiguousarray(np.asarray(x, dtype=np.float32))
    w1 = np.asarray(w1, dtype=np.float32)
    w2 = np.asarray(w2, dtype=np.float32)
    b1 = np.asarray(b1, dtype=np.float32)
    b2 = np.asarray(b2, dtype=np.float32)

    if "nc" not in _CACHE:
        _CACHE["nc"] = _build()
    nc = _CACHE["nc"]

    # weight layout: w1t[ci, t*128+co] = w1[co, ci, ky, kx]; duplicated on
    # partitions 64:128 for the upper row-group. w2t likewise (full 128 rows).
    w1r = np.transpose(w1, (1, 2, 3, 0)).reshape(CIN, 9 * 128)  # ci,(ky kx co)
    # reorder to (t*128 + co): currently (ky,kx) major over co -> already t-major
    w1full = np.concatenate([w1r, w1r], axis=0)                  # [128, 1152]
    w2r = np.transpose(w2, (1, 2, 3, 0)).reshape(COUT, 9 * 128)

    in_maps = []
    for c in range(N_CORES):
        in_maps.append({
            "x": np.ascontiguousarray(x[c * PB:(c + 1) * PB]),
            "w1t": w1full,
            "w2t": w2r,
            "b1": b1.reshape(128, 1),
            "b2": b2.reshape(128, 1),
        })

    res = run_bass_kernel_spmd(nc, in_maps, core_ids=list(range(N_CORES)),
                               trace=TRACE)
    LAST_RESULT = res
    out = np.empty((B, COUT, HO, WO), dtype=np.float32)
    for c in range(N_CORES):
        out[c * PB:(c + 1) * PB] = res.results[c]["y"]
    return out
